# revision 50
# baseline (speedup 1.0000x reference)
"""YIN pitch Trainium2 kernel: P=80 band-matmul.

C[f,tau] = sum_n x[n]*x[n+tau]*[80f <= n <= 80f+132] on the tensor engine.
With 80-sample contraction tiles, HOP=80 divides the tile exactly: tile t
touches only frames {t-1, t} with a fixed per-tile mask (ones for frame t,
[s<53] for frame t-1).  The Hankel moving operand has 80 partition-shifted
rows (6.5 MB of fp8 DMA instead of the 128-row 10.6 MB), streamed in
progressive segments from a single fp8 bounce of x.

Selector slab entries live at col 1056 a + 33 w - (t - f) (window block
pitch 1056, matmul reads 32-col windows at stride 32).  The slab source
xpm[s, t] = x[80 t + s] is produced by PE transposes of shifted-row chunks
loaded straight from DRAM in f32 (the xbar transpose-DMA would serialize
the whole DMA stream, so it is avoided).

Frames accumulate in per-window [32, W] PSUM tiles (matmul outputs must
sit at PSUM partition base 0); Act retires each stopped window into a
[128, W] block tile with the -e1/2 bias folded in, so the CMNDF chain is
d = -2*csb + e2, cum-scan, threshold pick (DVE with Pool assists).
The whole pipeline runs from one fp8 copy of x (3x threshold margin).
"""

import numpy as np

import bass_rust
import concourse.bass as bass
import concourse.mybir as mybir
import concourse.tile as tile
from concourse.bass_utils import run_bass_kernel_spmd

_WAIT_LIM = 1


def _split_excess_waits(nc):
    uid = 0
    for fn in nc.m.functions:
        for blk in fn.blocks:
            out = []
            changed = False
            for inst in blk.instructions:
                si = inst.sync_info
                waits = list(si.on_wait) if si is not None and si.on_wait else []
                if len(waits) > _WAIT_LIM:
                    changed = True
                    extra = waits[:-_WAIT_LIM]
                    si.on_wait = waits[-_WAIT_LIM:]
                    for j in range(0, len(extra), _WAIT_LIM):
                        nop = bass_rust.InstNoOp(name=f"WSPLIT-{uid}", ins=[], outs=[])
                        uid += 1
                        nop.engine = inst.engine
                        nop.sync_info = bass_rust.SyncInfo(
                            on_wait=extra[j:j + _WAIT_LIM], on_update=[]
                        )
                        out.append(nop)
                out.append(inst)
            if changed:
                blk.instructions = out


def _short_drain_and_barrier(self, tick_clock, wait_clock):
    # Tail with a single all-engine barrier: drain, barrier, sem cleanup.
    from concourse.vector_clock import ScopedClock
    nc = self.nc
    drain_inst = nc.sync.drain()
    wait_clock.add_sem_waits(
        drain_inst.ins, ScopedClock({None: tick_clock.global_clock})
    )
    nc.all_engine_barrier()
    assert self.sems is not None
    popped = nc._tile_sem_poison_stack.pop()
    assert popped is self._sem_poison
    nc.clear_and_free_semaphores(list(self.sems.allocated().values()))


tile.TileContext._drain_and_barrier = _short_drain_and_barrier


B = 8
N = 80000
SR = 8000
HOP = 80
TAU_MIN = 20
W = 133
FRAME_LEN = 266
N_OUT = 996
N_BLK = 8
BIG = 1.0e9

P = 80                   # contraction tile height (samples per tile)
NT = 1000                # sample tiles
NPAIR = 499              # DR pairs with live frames: u = 0..498
# progressive Hankel segments (tile counts must be even)
SEG_BOUNDS = [0, 160, 336, 512, 688, 800, 896, 944, 976, 992, 1000]

F32 = mybir.dt.float32
BF16 = mybir.dt.bfloat16
FP8 = mybir.dt.float8e4
AluOp = mybir.AluOpType
Axis = mybir.AxisListType
DR = mybir.MatmulPerfMode.DoubleRow


def _ap(t, offset, pairs):
    return bass.AP(t, offset, pairs)


def _sap(tile_ap, offset, pairs):
    """AP on an SBUF tile: partition pair step = row pitch (elements)."""
    pitch = tile_ap[:, 0:1].ap[0][0]
    return bass.AP(tile_ap.tensor, offset, [[pitch, pairs[0][1]]] + pairs[1:])


def _build_nc():
    nc = bass.Bass(trn_type="TRN2")
    x_d = nc.dram_tensor("x", [N], F32, kind="ExternalInput")
    f0_d = nc.dram_tensor("f0", [N_OUT], F32, kind="ExternalOutput")

    tau_row = np.arange(1, W + 1, dtype=np.float32)
    cpk_np = np.concatenate(
        [
            np.broadcast_to(5.0 * tau_row, (128, W)),       # 5*tau (thresh folded)
            np.broadcast_to(BIG + tau_row, (128, W)),       # BIG + tau
            np.eye(128, dtype=np.float32),                  # PE transpose identity
        ],
        axis=1,
    ).astype(np.float32)
    cpk_d = nc.inline_tensor(cpk_np, name="cpk")
    bm_np = np.zeros((96, 2), np.dtype(mybir.dt.np(BF16)))
    bm_np[:53, 0] = 1.0      # frame t-1 mask within tile t
    bm_np[:P, 1] = 1.0       # frame t mask (all 80 samples)
    bmask_d = nc.inline_tensor(bm_np, name="bmask")

    with tile.TileContext(nc) as tc:
        with (
            tc.tile_pool(name="persist", bufs=1) as pp,
            tc.tile_pool(name="work", bufs=2) as wp,
            tc.tile_pool(name="xdpool", bufs=6) as xdp,
            tc.tile_pool(name="psum", bufs=6, space="PSUM") as psp,
            tc.tile_pool(name="ptr", bufs=2, space="PSUM") as ptp,
            tc.tile_pool(name="dram", bufs=1, space="DRAM") as dp,
        ):
            # window block pitch 1056 cols (32-col unread gap per block);
            # tile w inside a block: entries at 33w (frame t) and 33w-1
            # (frame t-1); matmul reads 32-col windows at stride 32
            xb = pp.tile([P, 1056 * 32], FP8)
            bslab = pp.tile([P, 64 * 31], FP8)
            # split the zero fill across Pool/Act; DVE stays free for the
            # x -> fp8 convert chain that gates the DMA pipeline
            nc.gpsimd.memset(xb[:, 0:18176].bitcast(F32), 0.0)

            # ---- constants
            cpk = pp.tile([128, 2 * W + 128], F32)
            nc.scalar.dma_start(cpk[:], cpk_d[:])
            bmask = pp.tile([96, 2], BF16)
            nc.scalar.dma_start(bmask[:], bmask_d[:])
            tauc5 = cpk[:, 0:W]
            taubig = cpk[:, W:2 * W]
            ident = cpk[:, 2 * W:2 * W + 128]

            # ---- x -> SBUF chunk (f32), convert to fp8, bounce to DRAM
            xchunk = pp.tile([128, 640], F32)
            xlow = pp.tile([128, 640], FP8)
            nc.vector.memset(xlow[96:128, :].bitcast(F32), 0.0)
            nc.sync.dma_start(
                xchunk[0:125, :], _ap(x_d, 0, [[640, 125], [1, 640]])
            )
            nc.vector.tensor_copy(xlow[0:125, :], xchunk[0:125, :])

            nc.scalar.memzero(xb[:, 18176:33792].bitcast(F32))
            nc.vector.memset(bslab[:].bitcast(F32), 0.0)

            xpad8_d = dp.tile([130, 640], FP8)
            # first half gates Hankel segments for tiles < 512
            nc.sync.dma_start(xpad8_d[0:64, :], xlow[0:64, :])
            nc.sync.dma_start(xpad8_d[64:128, :], xlow[64:128, :])
            nc.sync.dma_start(
                _ap(xpad8_d.tensor, 81920, [[1, 640]]), xlow[127:128, :]
            )
            # xr32[t', 96 k + s] = x[80 (128 k + t') + s] straight from DRAM
            # (f32, k < 7); tail tiles t in [896, 1000) as a clipped block
            xr32 = pp.tile([128, 752], F32)
            nc.sync.dma_start(
                _sap(xr32, 0, [[1, 128], [96, 4], [1, 96]]),
                _ap(x_d, 0, [[80, 128], [10240, 4], [1, 96]]),
            )
            xpm = pp.tile([96, 1024], BF16)
            nc.vector.memset(xpm[0:96, 1000:1024].bitcast(F32), 0.0)

            def xpm_chunks(klo, khi):
                for k in range(klo, khi):
                    if k < 7:
                        pt = ptp.tile([96, 128], F32, tag="pt")
                        nc.tensor.transpose(
                            pt[:], xr32[:, 96 * k:96 * k + 96], ident
                        )
                        nc.vector.tensor_copy(
                            xpm[0:96, 128 * k:128 * k + 128], pt[:]
                        )
                    else:
                        pt = ptp.tile([96, 128], F32, tag="pt")
                        nc.tensor.transpose(
                            pt[0:80, 0:104], xr32[0:104, 672:752],
                            cpk[0:104, 2 * W:2 * W + 104],
                        )
                        nc.vector.tensor_copy(
                            xpm[0:80, 896:1000], pt[0:80, 0:104]
                        )

            # ---- slab build (DVE): entry (tile t = 32a + w, frame f) at
            # col 1056 a + 33 w - (t - f); frame-t cols for w = 0, then the
            # paired (frame t-1, frame t) runs for w in [1, 32).  Emitted in
            # two halves so matmuls start after the first 4 xpm chunks.
            def slab_half(alo, ahi):
                na = ahi - alo
                nc.vector.tensor_tensor(
                    out=_sap(xb, 1056 * alo, [[1, P], [1056, na], [1, 1]]),
                    in0=_sap(xpm, 32 * alo, [[1, P], [32, na], [0, 1]]),
                    in1=_sap(bmask, 1, [[1, P], [0, na], [1, 1]]),
                    op=AluOp.mult,
                )
                jlo, jhi = max(alo - 1, 0), ahi - 1
                nc.vector.tensor_tensor(
                    out=_sap(bslab, 64 * jlo + 31, [[1, P], [64, jhi - jlo], [1, 1]]),
                    in0=_sap(xpm, 32 * (jlo + 1), [[1, P], [32, jhi - jlo], [0, 1]]),
                    in1=_sap(bmask, 0, [[1, P], [0, jhi - jlo], [1, 1]]),
                    op=AluOp.mult,
                )
                for par in (0, 1):          # a parity
                    ilo, ihi = alo // 2, ahi // 2
                    ni = ihi - ilo
                    base = 2112 * ilo + 1056 * par + 32
                    nc.vector.tensor_tensor(
                        out=_sap(xb, base, [[1, P], [2112, ni], [33, 31], [1, 2]]),
                        in0=_sap(xpm, 64 * ilo + 32 * par + 1,
                                 [[1, P], [64, ni], [1, 31], [0, 2]]),
                        in1=_sap(bmask, 0, [[1, P], [0, ni], [0, 31], [1, 2]]),
                        op=AluOp.mult,
                    )

            xpm_chunks(0, 4)
            slab_half(0, 16)

            # ---- energy path: frames from the fp8 bounce; squares/scans are
            # emitted lazily per block inside the segment loop
            xfrall = pp.tile([128, 8 * FRAME_LEN], FP8)
            sq = {}
            qq = {}
            e1h = {}
            e12 = {}
            for b in range(N_BLK):
                sq[b] = pp.tile([128, FRAME_LEN], F32, name=f"sq{b}")
                qq[b] = pp.tile([128, FRAME_LEN], F32, name=f"qq{b}")
                e1h[b] = pp.tile([128, 1], F32, name=f"e1h{b}")
                e12[b] = pp.tile([128, W], F32, name=f"e12{b}")

            def energy_block(b):
                nc.scalar.square(
                    sq[b][:], xfrall[:, FRAME_LEN * b:FRAME_LEN * (b + 1)]
                )
                nc.vector.tensor_tensor_scan(
                    qq[b][:], sq[b][:], sq[b][:], 0.0, AluOp.add, AluOp.bypass
                )
                # -e1/2: folded into the window retire as an Act bias
                nc.vector.tensor_scalar(
                    out=e1h[b][:], in0=qq[b][:, W - 1:W], scalar1=-0.5,
                    scalar2=None, op0=AluOp.mult,
                )
                nc.gpsimd.tensor_sub(
                    e12[b][:], qq[b][:, W:FRAME_LEN], qq[b][:, 0:W]
                )

            f0all = pp.tile([128, N_BLK], F32)

            # ---- PSUM: one [32, W] tile per window (matmul out must sit at
            # PSUM partition base 0); Act copies stopped windows into csb.
            cps = {}
            csb = [pp.tile([128, W], F32, name=f"csb{b}") for b in range(N_BLK)]

            def _pwin(a):
                if a not in cps:
                    cps[a] = psp.tile([32, W], F32, tag="c", name=f"c{a}")
                return cps[a][:]

            def retire_window(a):
                b, q = a // 4, a % 4
                # csb = C - e1/2  (so d = -2*csb + e2 = e1 + e2 - 2C)
                nc.scalar.add(
                    csb[b][32 * q:32 * q + 32, :], cps[a][:],
                    e1h[b][32 * q:32 * q + 32, :],
                )
                del cps[a]

            def finish_block(b):
                d = wp.tile([128, W], F32, tag="d")
                nc.vector.scalar_tensor_tensor(
                    out=d[:], in0=csb[b][:], scalar=-2.0, in1=e12[b][:],
                    op0=AluOp.mult, op1=AluOp.add,
                )
                cum = wp.tile([128, W], F32, tag="cum")
                nc.vector.tensor_tensor_scan(
                    cum[:], d[:], d[:], 0.0, AluOp.add, AluOp.bypass
                )
                # lhs on Pool runs in parallel with the cum scan on DVE
                lhs = wp.tile([128, W], F32, tag="lhs")
                nc.gpsimd.tensor_mul(lhs[:], d[:], tauc5)
                cand = wp.tile([128, W], F32, tag="cand")
                nc.vector.tensor_tensor(
                    out=cand[:], in0=lhs[:], in1=cum[:], op=AluOp.is_lt
                )
                v = wp.tile([128, W], F32, tag="v")
                nc.vector.scalar_tensor_tensor(
                    out=v[:], in0=cand[:], scalar=-BIG, in1=taubig,
                    op0=AluOp.mult, op1=AluOp.add,
                )
                tmin = wp.tile([128, 1], F32, tag="tmin")
                nc.vector.tensor_reduce(
                    tmin[:], v[:, TAU_MIN - 1:W], axis=Axis.X, op=AluOp.min
                )
                voi = wp.tile([128, 1], F32, tag="voi")
                nc.vector.tensor_scalar(
                    out=voi[:], in0=tmin[:], scalar1=BIG * 0.5,
                    scalar2=None, op0=AluOp.is_lt,
                )
                rec = wp.tile([128, 1], F32, tag="rec")
                nc.vector.reciprocal(rec[:], tmin[:])
                nc.vector.scalar_tensor_tensor(
                    out=f0all[:, b:b + 1], in0=voi[:], scalar=float(SR),
                    in1=rec[:], op0=AluOp.mult, op1=AluOp.mult,
                )

            # ---- band matmuls over Hankel segments
            def late_setup():
                nc.sync.dma_start(
                    _sap(xr32, 384, [[1, 128], [96, 3], [1, 96]]),
                    _ap(x_d, 10240 * 4, [[80, 128], [10240, 3], [1, 96]]),
                )
                nc.sync.dma_start(
                    _sap(xr32, 672, [[1, 104], [1, 80]]),
                    _ap(x_d, 71680, [[80, 104], [1, 80]]),
                )
                xpm_chunks(4, 8)
                slab_half(16, 32)
                nc.sync.dma_start(
                    _sap(xfrall, 0, [[1, 128], [FRAME_LEN, 8], [1, FRAME_LEN]]),
                    _ap(xpad8_d.tensor, 0,
                        [[HOP, 128], [HOP * 128, 8], [1, FRAME_LEN]]),
                )

            for si in range(len(SEG_BOUNDS) - 1):
                t0, t1 = SEG_BOUNDS[si], SEG_BOUNDS[si + 1]
                seg_len = P * (t1 - t0) + 54
                xd = xdp.tile([P, P * 192 + 54], FP8, tag="xd")
                nc.sync.dma_start(
                    xd[:, 0:seg_len],
                    _ap(xpad8_d.tensor, P * t0, [[1, P], [1, seg_len]]),
                )
                for u in range(t0 // 2, t1 // 2):
                    if u >= NPAIR:
                        break
                    if u % 64 == 0 and u // 64 < N_BLK:
                        energy_block(u // 64)
                    off2 = 160 * u - P * t0
                    mv = _sap(xd, off2 + 1, [[1, P], [P, 2], [1, W]])
                    a = (2 * u + 1) // 32
                    if u > 0 and (2 * u) % 32 == 0:
                        nc.tensor.matmul(
                            _pwin(a - 1),
                            _sap(bslab, 64 * (a - 1), [[1, P], [32, 2], [1, 32]]),
                            mv,
                            start=False, stop=True,
                            perf_mode=DR, skip_group_check=True,
                        )
                        retire_window(a - 1)
                    nc.tensor.matmul(
                        _pwin(a),
                        _sap(xb, 1056 * a + 64 * (u - 16 * a),
                             [[1, P], [32, 2], [1, 32]]),
                        mv,
                        start=(u == 16 * a), stop=(u == NPAIR - 1),
                        perf_mode=DR, skip_group_check=True,
                    )
                    if u > 0 and (2 * u) % 128 == 0:
                        finish_block(u // 64 - 1)
                if si == 0:
                    late_setup()
            retire_window(31)
            finish_block(7)

            # ---- output: strided DMA straight from f0all
            nc.sync.dma_start(
                _ap(f0_d, 0, [[1, 128], [128, 7]]),
                _sap(f0all, 0, [[1, 128], [1, 7]]),
            )
            nc.sync.dma_start(
                _ap(f0_d, 896, [[1, 100]]), f0all[0:100, 7:8]
            )

    _split_excess_waits(nc)
    return nc


_NC_CACHE = {}


def _get_nc():
    if "nc" not in _NC_CACHE:
        _NC_CACHE["nc"] = _build_nc()
    return _NC_CACHE["nc"]


def kernel(x: np.ndarray) -> np.ndarray:
    x = np.ascontiguousarray(np.asarray(x), dtype=np.float32)
    assert x.shape == (B, N), x.shape
    nc = _get_nc()
    in_maps = [{"x": x[i]} for i in range(B)]
    res = run_bass_kernel_spmd(nc, in_maps, core_ids=list(range(B)))
    out = np.stack([np.asarray(res.results[i]["f0"]).reshape(N_OUT) for i in range(B)])
    return out.astype(np.float32)


# revision 51
# speedup vs baseline: 1.0025x; 1.0025x over previous
"""YIN pitch Trainium2 kernel: P=80 band-matmul.

C[f,tau] = sum_n x[n]*x[n+tau]*[80f <= n <= 80f+132] on the tensor engine.
With 80-sample contraction tiles, HOP=80 divides the tile exactly: tile t
touches only frames {t-1, t} with a fixed per-tile mask (ones for frame t,
[s<53] for frame t-1).  The Hankel moving operand has 80 partition-shifted
rows (6.5 MB of fp8 DMA instead of the 128-row 10.6 MB), streamed in
progressive segments from a single fp8 bounce of x.

Selector slab entries live at col 1056 a + 33 w - (t - f) (window block
pitch 1056, matmul reads 32-col windows at stride 32).  The slab source
xpm[s, t] = x[80 t + s] is produced by PE transposes of shifted-row chunks
loaded straight from DRAM in f32 (the xbar transpose-DMA would serialize
the whole DMA stream, so it is avoided).

Frames accumulate in per-window [32, W] PSUM tiles (matmul outputs must
sit at PSUM partition base 0); Act retires each stopped window into a
[128, W] block tile with the -e1/2 bias folded in, so the CMNDF chain is
d = -2*csb + e2, cum-scan, threshold pick (DVE with Pool assists).
The whole pipeline runs from one fp8 copy of x (3x threshold margin).
"""

import numpy as np

import bass_rust
import concourse.bass as bass
import concourse.mybir as mybir
import concourse.tile as tile
from concourse.bass_utils import run_bass_kernel_spmd

_WAIT_LIM = 1


def _split_excess_waits(nc):
    uid = 0
    for fn in nc.m.functions:
        for blk in fn.blocks:
            out = []
            changed = False
            for inst in blk.instructions:
                si = inst.sync_info
                waits = list(si.on_wait) if si is not None and si.on_wait else []
                if len(waits) > _WAIT_LIM:
                    changed = True
                    extra = waits[:-_WAIT_LIM]
                    si.on_wait = waits[-_WAIT_LIM:]
                    for j in range(0, len(extra), _WAIT_LIM):
                        nop = bass_rust.InstNoOp(name=f"WSPLIT-{uid}", ins=[], outs=[])
                        uid += 1
                        nop.engine = inst.engine
                        nop.sync_info = bass_rust.SyncInfo(
                            on_wait=extra[j:j + _WAIT_LIM], on_update=[]
                        )
                        out.append(nop)
                out.append(inst)
            if changed:
                blk.instructions = out


def _short_drain_and_barrier(self, tick_clock, wait_clock):
    # Tail with a single all-engine barrier: drain, barrier, sem cleanup.
    from concourse.vector_clock import ScopedClock
    nc = self.nc
    drain_inst = nc.sync.drain()
    wait_clock.add_sem_waits(
        drain_inst.ins, ScopedClock({None: tick_clock.global_clock})
    )
    nc.all_engine_barrier()
    assert self.sems is not None
    popped = nc._tile_sem_poison_stack.pop()
    assert popped is self._sem_poison
    nc.clear_and_free_semaphores(list(self.sems.allocated().values()))


tile.TileContext._drain_and_barrier = _short_drain_and_barrier


B = 8
N = 80000
SR = 8000
HOP = 80
TAU_MIN = 20
W = 133
FRAME_LEN = 266
N_OUT = 996
N_BLK = 8
BIG = 1.0e9

P = 80                   # contraction tile height (samples per tile)
NT = 1000                # sample tiles
NPAIR = 499              # DR pairs with live frames: u = 0..498
# progressive Hankel segments (tile counts must be even)
SEG_BOUNDS = [0, 128, 304, 480, 656, 784, 896, 944, 976, 992, 1000]

F32 = mybir.dt.float32
BF16 = mybir.dt.bfloat16
FP8 = mybir.dt.float8e4
AluOp = mybir.AluOpType
Axis = mybir.AxisListType
DR = mybir.MatmulPerfMode.DoubleRow


def _ap(t, offset, pairs):
    return bass.AP(t, offset, pairs)


def _sap(tile_ap, offset, pairs):
    """AP on an SBUF tile: partition pair step = row pitch (elements)."""
    pitch = tile_ap[:, 0:1].ap[0][0]
    return bass.AP(tile_ap.tensor, offset, [[pitch, pairs[0][1]]] + pairs[1:])


def _build_nc():
    nc = bass.Bass(trn_type="TRN2")
    x_d = nc.dram_tensor("x", [N], F32, kind="ExternalInput")
    f0_d = nc.dram_tensor("f0", [N_OUT], F32, kind="ExternalOutput")

    tau_row = np.arange(1, W + 1, dtype=np.float32)
    cpk_np = np.concatenate(
        [
            np.broadcast_to(5.0 * tau_row, (128, W)),       # 5*tau (thresh folded)
            np.broadcast_to(BIG + tau_row, (128, W)),       # BIG + tau
            np.eye(128, dtype=np.float32),                  # PE transpose identity
        ],
        axis=1,
    ).astype(np.float32)
    cpk_d = nc.inline_tensor(cpk_np, name="cpk")
    bm_np = np.zeros((96, 2), np.dtype(mybir.dt.np(BF16)))
    bm_np[:53, 0] = 1.0      # frame t-1 mask within tile t
    bm_np[:P, 1] = 1.0       # frame t mask (all 80 samples)
    bmask_d = nc.inline_tensor(bm_np, name="bmask")

    with tile.TileContext(nc) as tc:
        with (
            tc.tile_pool(name="persist", bufs=1) as pp,
            tc.tile_pool(name="work", bufs=2) as wp,
            tc.tile_pool(name="xdpool", bufs=6) as xdp,
            tc.tile_pool(name="psum", bufs=6, space="PSUM") as psp,
            tc.tile_pool(name="ptr", bufs=2, space="PSUM") as ptp,
            tc.tile_pool(name="dram", bufs=1, space="DRAM") as dp,
        ):
            # window block pitch 1056 cols (32-col unread gap per block);
            # tile w inside a block: entries at 33w (frame t) and 33w-1
            # (frame t-1); matmul reads 32-col windows at stride 32
            xb = pp.tile([P, 1056 * 32], FP8)
            bslab = pp.tile([P, 64 * 31], FP8)
            # split the zero fill across Pool/Act; DVE stays free for the
            # x -> fp8 convert chain that gates the DMA pipeline
            nc.gpsimd.memset(xb[:, 0:18176].bitcast(F32), 0.0)

            # ---- constants
            cpk = pp.tile([128, 2 * W + 128], F32)
            nc.scalar.dma_start(cpk[:], cpk_d[:])
            bmask = pp.tile([96, 2], BF16)
            nc.scalar.dma_start(bmask[:], bmask_d[:])
            tauc5 = cpk[:, 0:W]
            taubig = cpk[:, W:2 * W]
            ident = cpk[:, 2 * W:2 * W + 128]

            # ---- x -> SBUF chunk (f32), convert to fp8, bounce to DRAM
            xchunk = pp.tile([128, 640], F32)
            xlow = pp.tile([128, 640], FP8)
            nc.vector.memset(xlow[96:128, :].bitcast(F32), 0.0)
            nc.sync.dma_start(
                xchunk[0:125, :], _ap(x_d, 0, [[640, 125], [1, 640]])
            )
            nc.vector.tensor_copy(xlow[0:125, :], xchunk[0:125, :])

            nc.scalar.memzero(xb[:, 18176:33792].bitcast(F32))
            nc.vector.memset(bslab[:].bitcast(F32), 0.0)

            xpad8_d = dp.tile([130, 640], FP8)
            # first half gates Hankel segments for tiles < 512
            nc.sync.dma_start(xpad8_d[0:64, :], xlow[0:64, :])
            nc.sync.dma_start(xpad8_d[64:128, :], xlow[64:128, :])
            nc.sync.dma_start(
                _ap(xpad8_d.tensor, 81920, [[1, 640]]), xlow[127:128, :]
            )
            # xr32[t', 96 k + s] = x[80 (128 k + t') + s] straight from DRAM
            # (f32, k < 7); tail tiles t in [896, 1000) as a clipped block
            xr32 = pp.tile([128, 752], F32)
            nc.sync.dma_start(
                _sap(xr32, 0, [[1, 128], [96, 4], [1, 96]]),
                _ap(x_d, 0, [[80, 128], [10240, 4], [1, 96]]),
            )
            xpm = pp.tile([96, 1024], BF16)
            nc.vector.memset(xpm[0:96, 1000:1024].bitcast(F32), 0.0)

            def xpm_chunks(klo, khi):
                for k in range(klo, khi):
                    if k < 7:
                        pt = ptp.tile([96, 128], F32, tag="pt")
                        nc.tensor.transpose(
                            pt[:], xr32[:, 96 * k:96 * k + 96], ident
                        )
                        nc.vector.tensor_copy(
                            xpm[0:96, 128 * k:128 * k + 128], pt[:]
                        )
                    else:
                        pt = ptp.tile([96, 128], F32, tag="pt")
                        nc.tensor.transpose(
                            pt[0:80, 0:104], xr32[0:104, 672:752],
                            cpk[0:104, 2 * W:2 * W + 104],
                        )
                        nc.vector.tensor_copy(
                            xpm[0:80, 896:1000], pt[0:80, 0:104]
                        )

            # ---- slab build (DVE): entry (tile t = 32a + w, frame f) at
            # col 1056 a + 33 w - (t - f); frame-t cols for w = 0, then the
            # paired (frame t-1, frame t) runs for w in [1, 32).  Emitted in
            # two halves so matmuls start after the first 4 xpm chunks.
            def slab_half(alo, ahi):
                na = ahi - alo
                nc.vector.tensor_tensor(
                    out=_sap(xb, 1056 * alo, [[1, P], [1056, na], [1, 1]]),
                    in0=_sap(xpm, 32 * alo, [[1, P], [32, na], [0, 1]]),
                    in1=_sap(bmask, 1, [[1, P], [0, na], [1, 1]]),
                    op=AluOp.mult,
                )
                jlo, jhi = max(alo - 1, 0), ahi - 1
                nc.vector.tensor_tensor(
                    out=_sap(bslab, 64 * jlo + 31, [[1, P], [64, jhi - jlo], [1, 1]]),
                    in0=_sap(xpm, 32 * (jlo + 1), [[1, P], [32, jhi - jlo], [0, 1]]),
                    in1=_sap(bmask, 0, [[1, P], [0, jhi - jlo], [1, 1]]),
                    op=AluOp.mult,
                )
                for par in (0, 1):          # a parity
                    ilo, ihi = alo // 2, ahi // 2
                    ni = ihi - ilo
                    base = 2112 * ilo + 1056 * par + 32
                    nc.vector.tensor_tensor(
                        out=_sap(xb, base, [[1, P], [2112, ni], [33, 31], [1, 2]]),
                        in0=_sap(xpm, 64 * ilo + 32 * par + 1,
                                 [[1, P], [64, ni], [1, 31], [0, 2]]),
                        in1=_sap(bmask, 0, [[1, P], [0, ni], [0, 31], [1, 2]]),
                        op=AluOp.mult,
                    )

            xpm_chunks(0, 4)
            slab_half(0, 16)

            # ---- energy path: frames from the fp8 bounce; squares/scans are
            # emitted lazily per block inside the segment loop
            xfrall = pp.tile([128, 8 * FRAME_LEN], FP8)
            sq = {}
            qq = {}
            e1h = {}
            e12 = {}
            for b in range(N_BLK):
                sq[b] = pp.tile([128, FRAME_LEN], F32, name=f"sq{b}")
                qq[b] = pp.tile([128, FRAME_LEN], F32, name=f"qq{b}")
                e1h[b] = pp.tile([128, 1], F32, name=f"e1h{b}")
                e12[b] = pp.tile([128, W], F32, name=f"e12{b}")

            def energy_block(b):
                nc.scalar.square(
                    sq[b][:], xfrall[:, FRAME_LEN * b:FRAME_LEN * (b + 1)]
                )
                nc.vector.tensor_tensor_scan(
                    qq[b][:], sq[b][:], sq[b][:], 0.0, AluOp.add, AluOp.bypass
                )
                # -e1/2: folded into the window retire as an Act bias
                nc.vector.tensor_scalar(
                    out=e1h[b][:], in0=qq[b][:, W - 1:W], scalar1=-0.5,
                    scalar2=None, op0=AluOp.mult,
                )
                nc.gpsimd.tensor_sub(
                    e12[b][:], qq[b][:, W:FRAME_LEN], qq[b][:, 0:W]
                )

            f0all = pp.tile([128, N_BLK], F32)

            # ---- PSUM: one [32, W] tile per window (matmul out must sit at
            # PSUM partition base 0); Act copies stopped windows into csb.
            cps = {}
            csb = [pp.tile([128, W], F32, name=f"csb{b}") for b in range(N_BLK)]

            def _pwin(a):
                if a not in cps:
                    cps[a] = psp.tile([32, W], F32, tag="c", name=f"c{a}")
                return cps[a][:]

            def retire_window(a):
                b, q = a // 4, a % 4
                # csb = C - e1/2  (so d = -2*csb + e2 = e1 + e2 - 2C)
                nc.scalar.add(
                    csb[b][32 * q:32 * q + 32, :], cps[a][:],
                    e1h[b][32 * q:32 * q + 32, :],
                )
                del cps[a]

            def finish_block(b):
                d = wp.tile([128, W], F32, tag="d")
                nc.vector.scalar_tensor_tensor(
                    out=d[:], in0=csb[b][:], scalar=-2.0, in1=e12[b][:],
                    op0=AluOp.mult, op1=AluOp.add,
                )
                cum = wp.tile([128, W], F32, tag="cum")
                nc.vector.tensor_tensor_scan(
                    cum[:], d[:], d[:], 0.0, AluOp.add, AluOp.bypass
                )
                # lhs on Pool runs in parallel with the cum scan on DVE
                lhs = wp.tile([128, W], F32, tag="lhs")
                nc.gpsimd.tensor_mul(lhs[:], d[:], tauc5)
                cand = wp.tile([128, W], F32, tag="cand")
                nc.vector.tensor_tensor(
                    out=cand[:], in0=lhs[:], in1=cum[:], op=AluOp.is_lt
                )
                v = wp.tile([128, W], F32, tag="v")
                nc.vector.scalar_tensor_tensor(
                    out=v[:], in0=cand[:], scalar=-BIG, in1=taubig,
                    op0=AluOp.mult, op1=AluOp.add,
                )
                tmin = wp.tile([128, 1], F32, tag="tmin")
                nc.vector.tensor_reduce(
                    tmin[:], v[:, TAU_MIN - 1:W], axis=Axis.X, op=AluOp.min
                )
                voi = wp.tile([128, 1], F32, tag="voi")
                nc.vector.tensor_scalar(
                    out=voi[:], in0=tmin[:], scalar1=BIG * 0.5,
                    scalar2=None, op0=AluOp.is_lt,
                )
                rec = wp.tile([128, 1], F32, tag="rec")
                nc.vector.reciprocal(rec[:], tmin[:])
                nc.vector.scalar_tensor_tensor(
                    out=f0all[:, b:b + 1], in0=voi[:], scalar=float(SR),
                    in1=rec[:], op0=AluOp.mult, op1=AluOp.mult,
                )

            # ---- band matmuls over Hankel segments
            def late_setup():
                nc.sync.dma_start(
                    _sap(xr32, 384, [[1, 128], [96, 3], [1, 96]]),
                    _ap(x_d, 10240 * 4, [[80, 128], [10240, 3], [1, 96]]),
                )
                nc.sync.dma_start(
                    _sap(xr32, 672, [[1, 104], [1, 80]]),
                    _ap(x_d, 71680, [[80, 104], [1, 80]]),
                )
                xpm_chunks(4, 8)
                slab_half(16, 32)
                nc.sync.dma_start(
                    _sap(xfrall, 0, [[1, 128], [FRAME_LEN, 8], [1, FRAME_LEN]]),
                    _ap(xpad8_d.tensor, 0,
                        [[HOP, 128], [HOP * 128, 8], [1, FRAME_LEN]]),
                )

            for si in range(len(SEG_BOUNDS) - 1):
                t0, t1 = SEG_BOUNDS[si], SEG_BOUNDS[si + 1]
                seg_len = P * (t1 - t0) + 54
                xd = xdp.tile([P, P * 192 + 54], FP8, tag="xd")
                nc.sync.dma_start(
                    xd[:, 0:seg_len],
                    _ap(xpad8_d.tensor, P * t0, [[1, P], [1, seg_len]]),
                )
                for u in range(t0 // 2, t1 // 2):
                    if u >= NPAIR:
                        break
                    if u % 64 == 0 and u // 64 < N_BLK:
                        energy_block(u // 64)
                    off2 = 160 * u - P * t0
                    mv = _sap(xd, off2 + 1, [[1, P], [P, 2], [1, W]])
                    a = (2 * u + 1) // 32
                    if u > 0 and (2 * u) % 32 == 0:
                        nc.tensor.matmul(
                            _pwin(a - 1),
                            _sap(bslab, 64 * (a - 1), [[1, P], [32, 2], [1, 32]]),
                            mv,
                            start=False, stop=True,
                            perf_mode=DR, skip_group_check=True,
                        )
                        retire_window(a - 1)
                    nc.tensor.matmul(
                        _pwin(a),
                        _sap(xb, 1056 * a + 64 * (u - 16 * a),
                             [[1, P], [32, 2], [1, 32]]),
                        mv,
                        start=(u == 16 * a), stop=(u == NPAIR - 1),
                        perf_mode=DR, skip_group_check=True,
                    )
                    if u > 0 and (2 * u) % 128 == 0:
                        finish_block(u // 64 - 1)
                if si == 0:
                    late_setup()
            retire_window(31)
            finish_block(7)

            # ---- output: strided DMA straight from f0all
            nc.sync.dma_start(
                _ap(f0_d, 0, [[1, 128], [128, 7]]),
                _sap(f0all, 0, [[1, 128], [1, 7]]),
            )
            nc.sync.dma_start(
                _ap(f0_d, 896, [[1, 100]]), f0all[0:100, 7:8]
            )

    _split_excess_waits(nc)
    return nc


_NC_CACHE = {}


def _get_nc():
    if "nc" not in _NC_CACHE:
        _NC_CACHE["nc"] = _build_nc()
    return _NC_CACHE["nc"]


def kernel(x: np.ndarray) -> np.ndarray:
    x = np.ascontiguousarray(np.asarray(x), dtype=np.float32)
    assert x.shape == (B, N), x.shape
    nc = _get_nc()
    in_maps = [{"x": x[i]} for i in range(B)]
    res = run_bass_kernel_spmd(nc, in_maps, core_ids=list(range(B)))
    out = np.stack([np.asarray(res.results[i]["f0"]).reshape(N_OUT) for i in range(B)])
    return out.astype(np.float32)
